# revision 1
# baseline (speedup 1.0000x reference)
"""RWKV block (T=8192, D=2048) on 8 Trainium2 NeuronCores.

Strategy: data-parallel over the sequence (1024 tokens/core) with a
256-token recomputed warmup prefix per core (power-decay attention forgets
at e^{-|w|} per step, |w|>=0.6, so 256 steps of history reproduce the true
WKV state to ~e^{-150} — exact at fp32). Everything runs feature-major
[D partitions, T free]: token-shift is a free-axis offset, LayerNorm stats
are ones-matmuls on the tensor engine, and the WKV recurrence is two
`tensor_tensor_scan` linear scans (state = e^w * state + e^k [*v]) which are
numerically safe unstabilized because k ~ N(0,1). GEMMs run in fp32r
(full-rate on the PE). LN gamma/beta are folded into the following GEMM
weights on the host; weights are host-transposed to [d_in, d_out].
Core 0's warmup block is the wrapped tail x[T-256:], which both feeds the
roll-wrap token shifts and produces rz[T-1] for the FFN branch's wrap row;
a per-core mask input zeroes the scan carry at the warmup/main boundary so
core 0's main block starts from empty state like the reference.
"""
import sys
if '/opt/trn_rl_repo' not in sys.path:
    sys.path.insert(0, '/opt/trn_rl_repo')

from contextlib import ExitStack
import numpy as np

import concourse.bass as bass
import concourse.tile as tile
from concourse import bacc, mybir
from concourse.bass import _add_dep_helper
from concourse.bass_utils import run_bass_kernel_spmd

F32 = mybir.dt.float32
F32R = mybir.dt.float32r
AF = mybir.ActivationFunctionType
OP = mybir.AluOpType

D = 2048
T = 8192
NCORES = 8
TLOC = T // NCORES          # 1024 main tokens per core
H = 256                     # warmup tokens
TBUF = H + TLOC             # 1280
BS = 256                    # token block size
NB = TBUF // BS             # 5 blocks; block 0 is the warmup
NT = D // 128               # 16 partition tiles
JQ = 2                      # j-tiles per psum group (256 output channels)

WNAMES = ['wk', 'wv', 'wr', 'wo', 'wfk', 'wfv', 'wfr']
BNAMES = ['bk', 'bv', 'br', 'bo', 'bfk', 'bfv', 'bfr']
VNAMES = ['mixk', 'mixv', 'mixr', 'fmixk', 'fmixr', 'ew', 'eu', 'cmask']


def build_kernel():
    nc = bacc.Bacc()
    xT = nc.declare_dram_parameter('xT', [D, TBUF], F32R, isOutput=False)
    onescol = nc.declare_dram_parameter('onescol', [128, 1], F32R, isOutput=False)
    onesrow = nc.declare_dram_parameter('onesrow', [1, 128], F32R, isOutput=False)
    wd = {n: nc.declare_dram_parameter(n, [D, D], F32R, isOutput=False)
          for n in WNAMES}
    vd = {n: nc.declare_dram_parameter(n, [D, 1], F32, isOutput=False)
          for n in BNAMES + VNAMES}
    outT = nc.declare_dram_parameter('outT', [D, TLOC], F32, isOutput=True)

    xTt = xT.rearrange('(n p) t -> n p t', p=128)
    outTt = outT.rearrange('(n p) t -> n p t', p=128)
    wdt = {n: w.rearrange('(n p) j -> n p j', p=128) for n, w in wd.items()}
    # per-channel vectors as [128, NT]: col i holds channels [i*128, (i+1)*128)
    vdt = {n: v.rearrange('(n p) o -> p (n o)', p=128) for n, v in vd.items()}

    with tile.TileContext(nc) as tc:
        with ExitStack() as ctx:
            kern(ctx, tc, xTt, wdt, vdt, outTt, onescol, onesrow)
    nc.compile()
    return nc


def kern(ctx, tc, xTt, wdt, vdt, outTt, onescol, onesrow):
    nc = tc.nc

    cons = ctx.enter_context(tc.tile_pool(name='cons', bufs=1))
    cv = {}
    for n in BNAMES + VNAMES:
        cvt = cons.tile([128, NT], F32, tag=f'cv_{n}', name=f'cv_{n}')
        nc.sync.dma_start(cvt[:], vdt[n])
        cv[n] = [cvt[:, i:i + 1] for i in range(NT)]
    ones = cons.tile([128, 1], F32R, tag='ones', name='ones')
    nc.sync.dma_start(ones[:], onescol[:])
    ones_row = cons.tile([1, 128], F32R, tag='ones_row', name='ones_row')
    nc.sync.dma_start(ones_row[:], onesrow[:])

    # persistent boundary-state columns (scan carries / U lead cols)
    colp = ctx.enter_context(tc.tile_pool(name='colp', bufs=1))

    # weight panels [128, JQ*128] per K-tile, double buffered per tag
    wpool = ctx.enter_context(tc.tile_pool(name='wpool', bufs=2))
    # block activation tensors: one tag per (tensor, d-tile), single buffer;
    # Tile recycles a slot as soon as its last reader retires.
    ap = ctx.enter_context(tc.tile_pool(name='ap', bufs=64))     # shared slots
    ap2 = ctx.enter_context(tc.tile_pool(name='ap2', bufs=1))    # long-lived per-i
    wkvp = ctx.enter_context(tc.tile_pool(name='wkvp', bufs=2))  # per-i transients
    scr = ctx.enter_context(tc.tile_pool(name='scr', bufs=2))    # small scratch
    rows = ctx.enter_context(tc.tile_pool(name='rows', bufs=2))  # [1,BS] stats
    psg = ctx.enter_context(tc.tile_pool(name='psg', bufs=6, space='PSUM'))
    pss = ctx.enter_context(tc.tile_pool(name='pss', bufs=1, space='PSUM'))

    def pe_guard(aps):
        """Fused-LDW fp32/fp32r matmuls can carry only ONE sync wait in the
        ISA. Emit a PE NoOp that *reads* the given APs: Tile's dependency
        tracker assigns all cross-engine waits to it through the normal
        wait-clock path, so matmuls ordered behind it on the PE queue
        inherit the observed clocks (waits elided). The APs are stripped
        from the NoOp at lowering (framework-supported sync idiom)."""
        eng = nc.tensor
        inst = mybir.InstNoOp(
            name=nc.get_next_instruction_name(),
            text_hint='pe_guard', bass_nofuse=True,
            ins=[eng.lower_ap(a) for a in aps])
        return eng.add_instruction(inst)

    def gemm(wname, rhs, rhs_insts, out_cb):
        """out[j, :] = sum_d w[d, j] * rhs[d], contraction over all of D.
        rhs: list of 16 fp32r APs [128, BS]. out_cb(jt, psum_ap)."""
        for j0 in range(0, NT, JQ):
            panels = [wpool.tile([128, JQ * 128], F32R, tag=f'w_{kt}', name=f'w_{kt}')
                      for kt in range(NT)]
            for kt in range(NT):
                nc.sync.dma_start(panels[kt][:],
                                  wdt[wname][kt, :, j0 * 128:(j0 + JQ) * 128])
            guard = pe_guard([p[:] for p in panels] + list(rhs))
            for jj in range(JQ):
                pt = psg.tile([128, BS], F32, tag='ps', name='ps')
                for kt in range(NT):
                    mm = nc.tensor.matmul(
                        pt[:], panels[kt][:, jj * 128:(jj + 1) * 128],
                        rhs[kt], start=(kt == 0), stop=(kt == NT - 1))
                    _add_dep_helper(mm.ins, guard.ins, sync=False,
                                    reason='order after guard')
                out_cb(j0 + jj, pt[:])

    def ln_stats(xtiles, xinsts, tagp):
        """Per-token mean/rstd over the partition axis via ones-matmuls.
        xtiles: 16 fp32r APs [128, BS]. Returns (s_b, ms_b) [128, BS]."""
        ps_s = pss.tile([1, BS], F32, tag='st0', name='st0')
        ps_q = pss.tile([1, BS], F32, tag='st1', name='st1')
        sq0 = scr.tile([128, BS], F32R, tag='sq', name='sq', bufs=4)
        nc.scalar.activation(sq0[:], xtiles[0], AF.Square)
        guard = pe_guard(list(xtiles) + [sq0[:], ones[:]])
        for kt in range(NT):
            if kt == 0:
                sq = sq0
            else:
                sq = scr.tile([128, BS], F32R, tag='sq', name='sq', bufs=4)
                nc.scalar.activation(sq[:], xtiles[kt], AF.Square)
            mm = nc.tensor.matmul(ps_s[:], ones[:], xtiles[kt],
                                  start=(kt == 0), stop=(kt == NT - 1))
            _add_dep_helper(mm.ins, guard.ins, sync=False, reason='g')
            mm2 = nc.tensor.matmul(ps_q[:], ones[:], sq[:],
                                   start=(kt == 0), stop=(kt == NT - 1))
            _add_dep_helper(mm2.ins, guard.ins, sync=False, reason='g')
        mean = rows.tile([1, BS], F32, tag='mean', name='mean')
        var = rows.tile([1, BS], F32, tag='var', name='var')
        m2 = rows.tile([1, BS], F32, tag='m2', name='m2')
        nc.vector.tensor_scalar_mul(mean[:], ps_s[:], 1.0 / D)
        nc.vector.tensor_scalar_mul(var[:], ps_q[:], 1.0 / D)
        nc.vector.tensor_mul(m2[:], mean[:], mean[:])
        nc.vector.tensor_sub(var[:], var[:], m2[:])
        nc.vector.tensor_scalar_add(var[:], var[:], 1e-5)
        # rstd = exp(-0.5 * ln(var + eps))
        lnv = rows.tile([1, BS], F32, tag='lnv', name='lnv')
        nc.scalar.activation(lnv[:], var[:], AF.Ln)
        rstd = rows.tile([1, BS], F32R, tag='rstd', name='rstd')
        nc.scalar.activation(rstd[:], lnv[:], AF.Exp, scale=-0.5)
        ms = rows.tile([1, BS], F32R, tag='ms', name='ms')
        nc.vector.tensor_mul(ms[:], mean[:], rstd[:])
        # broadcast rows across partitions via K=1 ones-matmul into PSUM
        s_b = pss.tile([128, BS], F32, tag='st0', name='s_b')
        ms_b = pss.tile([128, BS], F32, tag='st1', name='ms_b')
        guard2 = pe_guard([rstd[:], ms[:], ones_row[:]])
        mmb = nc.tensor.matmul(s_b[:], ones_row[:], rstd[:], start=True, stop=True)
        _add_dep_helper(mmb.ins, guard2.ins, sync=False, reason='g2')
        mmb2 = nc.tensor.matmul(ms_b[:], ones_row[:], ms[:], start=True, stop=True)
        _add_dep_helper(mmb2.ins, guard2.ins, sync=False, reason='g2')
        return s_b, ms_b

    # persistent cross-block state
    Ucol = [colp.tile([128, 1], F32, tag=f'uc{i}', name=f'uc{i}') for i in range(NT)]
    U2col = [colp.tile([128, 1], F32, tag=f'u2c{i}', name=f'u2c{i}') for i in range(NT)]
    Acol = [colp.tile([128, 1], F32, tag=f'acl{i}', name=f'acl{i}') for i in range(NT)]
    Bcol = [colp.tile([128, 1], F32, tag=f'bcl{i}', name=f'bcl{i}') for i in range(NT)]
    for i in range(NT):
        nc.vector.memset(Ucol[i][:], 0.0)
        nc.vector.memset(U2col[i][:], 0.0)
        nc.vector.memset(Acol[i][:], 0.0)
        nc.vector.memset(Bcol[i][:], 0.0)

    for b in range(NB):
        t0 = b * BS
        ffn = b >= 1  # FFN & output for main blocks only

        # ---- x block + LN1 -> U ----
        xb = [ap.tile([128, BS], F32R, tag='blk', name=f'x{i}') for i in range(NT)]
        xdmas = [nc.sync.dma_start(xb[i][:], xTt[i, :, t0:t0 + BS])
                 for i in range(NT)]
        s_b, ms_b = ln_stats([xb[i][:] for i in range(NT)], xdmas, 'l1')
        U = [ap2.tile([128, BS + 1], F32, tag=f'u{i}', name=f'u{i}') for i in range(NT)]
        for i in range(NT):
            nc.vector.tensor_copy(U[i][:, 0:1], Ucol[i][:])
            t1 = scr.tile([128, BS], F32, tag='ut', name='ut')
            nc.vector.tensor_mul(t1[:], xb[i][:].bitcast(F32), s_b[:])
            nc.vector.tensor_sub(U[i][:, 1:BS + 1], t1[:], ms_b[:])
            nc.vector.tensor_copy(Ucol[i][:], U[i][:, BS:BS + 1])

        # ---- mixes (d1 shared) ----
        d1 = [ap.tile([128, BS], F32, tag='blk', name=f'd1_{i}') for i in range(NT)]
        for i in range(NT):
            nc.vector.tensor_sub(d1[i][:], U[i][:, 1:BS + 1], U[i][:, 0:BS])

        def mk_mix(mixname, tagc):
            mts = [ap.tile([128, BS], F32R, tag='blk', name=f'{tagc}{i}')
                   for i in range(NT)]
            insts = [nc.vector.scalar_tensor_tensor(
                mts[i][:], d1[i][:], cv[mixname][i],
                U[i][:, 0:BS], OP.mult, OP.add) for i in range(NT)]
            return mts, insts

        # ---- k / r / v GEMMs ----
        ink, inki = mk_mix('mixk', 'mk')
        EK = [ap.tile([128, BS], F32, tag='blk', name=f'ek{i}') for i in range(NT)]
        gemm('wk', [tl[:] for tl in ink], inki,
             lambda jt, ps: nc.scalar.activation(EK[jt][:], ps, AF.Exp,
                                                 bias=cv['bk'][jt]))
        inr, inri = mk_mix('mixr', 'mr')
        rsig = [ap.tile([128, BS], F32, tag='blk', name=f'rs{i}') for i in range(NT)]
        gemm('wr', [tl[:] for tl in inr], inri,
             lambda jt, ps: nc.scalar.activation(rsig[jt][:], ps, AF.Sigmoid,
                                                 bias=cv['br'][jt]))
        inv, invi = mk_mix('mixv', 'mv')
        vv = [ap.tile([128, BS], F32, tag='blk', name=f'vv{i}') for i in range(NT)]
        gemm('wv', [tl[:] for tl in inv], invi,
             lambda jt, ps: nc.scalar.activation(vv[jt][:], ps, AF.Identity,
                                                 bias=cv['bv'][jt]))

        # ---- WKV scans + assembly -> wkv_r ----
        wkvr = [ap.tile([128, BS], F32R, tag='blk', name=f'wr{i}') for i in range(NT)]
        wkvri = []
        for i in range(NT):
            EKV = wkvp.tile([128, BS], F32, tag='ekv', name='ekv')
            nc.vector.tensor_mul(EKV[:], EK[i][:], vv[i][:])
            A = wkvp.tile([128, BS + 1], F32, tag='A', name='A')
            B = wkvp.tile([128, BS + 1], F32, tag='B', name='B')
            if b == 1:
                # core-0 zeroes its carry here (cmask=0): the main range
                # starts from empty state like the reference's t=0
                nc.vector.tensor_mul(A[:, 0:1], Acol[i][:], cv['cmask'][i])
                nc.vector.tensor_mul(B[:, 0:1], Bcol[i][:], cv['cmask'][i])
            else:
                nc.vector.tensor_copy(A[:, 0:1], Acol[i][:])
                nc.vector.tensor_copy(B[:, 0:1], Bcol[i][:])
            ewb = cv['ew'][i].broadcast_to([128, BS])
            nc.vector.tensor_tensor_scan(A[:, 1:BS + 1], ewb, EKV[:],
                                         A[:, 0:1], OP.mult, OP.add)
            nc.vector.tensor_tensor_scan(B[:, 1:BS + 1], ewb, EK[i][:],
                                         B[:, 0:1], OP.mult, OP.add)
            nc.vector.tensor_copy(Acol[i][:], A[:, BS:BS + 1])
            nc.vector.tensor_copy(Bcol[i][:], B[:, BS:BS + 1])
            num = wkvp.tile([128, BS], F32, tag='num', name='num')
            den = wkvp.tile([128, BS], F32, tag='den', name='den')
            nc.vector.scalar_tensor_tensor(num[:], EKV[:], cv['eu'][i],
                                           A[:, 0:BS], OP.mult, OP.add)
            nc.vector.scalar_tensor_tensor(den[:], EK[i][:], cv['eu'][i],
                                           B[:, 0:BS], OP.mult, OP.add)
            rec = wkvp.tile([128, BS], F32, tag='rec', name='rec')
            nc.vector.reciprocal_approx_fast(rec[:], den[:])
            wkv = wkvp.tile([128, BS], F32, tag='wkv', name='wkv')
            nc.vector.tensor_mul(wkv[:], num[:], rec[:])
            wkvri.append(nc.vector.tensor_mul(wkvr[i][:], wkv[:], rsig[i][:]))

        # ---- atto-GEMM -> rz (x reloaded) ----
        x2 = [ap.tile([128, BS], F32, tag='blk', name=f'x2_{i}') for i in range(NT)]
        for i in range(NT):
            nc.sync.dma_start(x2[i][:], xTt[i, :, t0:t0 + BS].bitcast(F32))
        rz = [ap2.tile([128, BS], F32R, tag=f'rz{i}', name=f'rz{i}') for i in range(NT)]
        rzi = [None] * NT

        def o_cb(jt, ps):
            rzi[jt] = nc.vector.scalar_tensor_tensor(rz[jt][:], ps, cv['bo'][jt],
                                                     x2[jt][:], OP.add, OP.add)
        gemm('wo', [tl[:] for tl in wkvr], wkvri, o_cb)

        # ---- LN2 -> U2 ----
        s2b, ms2b = ln_stats([rz[i][:] for i in range(NT)], rzi, 'l2')
        U2 = [ap2.tile([128, BS + 1], F32, tag=f'w2{i}', name=f'w2{i}') for i in range(NT)]
        for i in range(NT):
            nc.vector.tensor_copy(U2[i][:, 0:1], U2col[i][:])
            t2 = scr.tile([128, BS], F32, tag='u2t', name='u2t')
            nc.vector.tensor_mul(t2[:], rz[i][:].bitcast(F32), s2b[:])
            nc.vector.tensor_sub(U2[i][:, 1:BS + 1], t2[:], ms2b[:])
            nc.vector.tensor_copy(U2col[i][:], U2[i][:, BS:BS + 1])

        if not ffn:
            continue

        # ---- FFN ----
        d2 = [ap.tile([128, BS], F32, tag='blk', name=f'e2_{i}') for i in range(NT)]
        for i in range(NT):
            nc.vector.tensor_sub(d2[i][:], U2[i][:, 1:BS + 1], U2[i][:, 0:BS])

        def mk_fmix(mixname, tagc):
            mts = [ap.tile([128, BS], F32R, tag='blk', name=f'{tagc}{i}')
                   for i in range(NT)]
            insts = [nc.vector.scalar_tensor_tensor(
                mts[i][:], d2[i][:], cv[mixname][i],
                U2[i][:, 0:BS], OP.mult, OP.add) for i in range(NT)]
            return mts, insts

        fki, fkii = mk_fmix('fmixk', 'fk')
        kf2 = [ap.tile([128, BS], F32R, tag='blk', name=f'kq{i}') for i in range(NT)]
        kf2i = [None] * NT

        def fk_cb(jt, ps):
            kf = scr.tile([128, BS], F32, tag='kf', name='kf')
            nc.scalar.activation(kf[:], ps, AF.Identity, bias=cv['bfk'][jt])
            # relu(kf)^2 == max(kf,0)*kf in one fused DVE op
            kf2i[jt] = nc.vector.scalar_tensor_tensor(kf2[jt][:], kf[:], 0.0,
                                                      kf[:], OP.max, OP.mult)
        gemm('wfk', [tl[:] for tl in fki], fkii, fk_cb)

        fri, frii = mk_fmix('fmixr', 'fr')
        rf = [ap.tile([128, BS], F32, tag='blk', name=f'rf{i}') for i in range(NT)]
        gemm('wfr', [tl[:] for tl in fri], frii,
             lambda jt, ps: nc.scalar.activation(rf[jt][:], ps, AF.Sigmoid,
                                                 bias=cv['bfr'][jt]))

        def fv_cb(jt, ps):
            t3 = scr.tile([128, BS], F32, tag='fo', name='fo')
            nc.vector.scalar_tensor_tensor(t3[:], ps, cv['bfv'][jt],
                                           rf[jt][:], OP.add, OP.mult)
            ot = scr.tile([128, BS], F32, tag='ot', name='ot')
            nc.vector.tensor_add(ot[:], t3[:], rz[jt][:].bitcast(F32))
            nc.sync.dma_start(outTt[jt, :, t0 - H:t0 - H + BS], ot[:])
        gemm('wfv', [tl[:] for tl in kf2], kf2i, fv_cb)


def prep_inputs(inputs):
    f32 = np.float32
    x = np.asarray(inputs['x'], f32)
    g1, b1 = np.asarray(inputs['ln1_g'], f32), np.asarray(inputs['ln1_b'], f32)
    g2, b2 = np.asarray(inputs['ln2_g'], f32), np.asarray(inputs['ln2_b'], f32)
    W, Bv = {}, {}
    for key, nm, g, b in [('wk', 'attk', g1, b1), ('wv', 'attv', g1, b1),
                          ('wr', 'attr', g1, b1), ('wfk', 'ffnk', g2, b2),
                          ('wfr', 'ffnr', g2, b2)]:
        w = np.asarray(inputs[nm + '_w'], f32)
        W[key] = np.ascontiguousarray((w * g[None, :]).T)
        Bv[key] = (np.asarray(inputs[nm + '_b'], f32) + w @ b).astype(f32)
    for key, nm in [('wo', 'atto'), ('wfv', 'ffnv')]:
        w = np.asarray(inputs[nm + '_w'], f32)
        W[key] = np.ascontiguousarray(w.T)
        Bv[key] = np.asarray(inputs[nm + '_b'], f32)
    bmap = dict(zip(BNAMES, ['wk', 'wv', 'wr', 'wo', 'wfk', 'wfv', 'wfr']))
    col = lambda a: np.ascontiguousarray(np.asarray(a, f32).reshape(D, 1))
    mixes = {'mixk': inputs['attmixk'], 'mixv': inputs['attmixv'],
             'mixr': inputs['attmixr'], 'fmixk': inputs['ffnmixk'],
             'fmixr': inputs['ffnmixr']}
    ew = np.exp(-np.exp(np.asarray(inputs['time_decay'], f32))).astype(f32)
    eu = np.exp(np.asarray(inputs['time_first'], f32)).astype(f32)
    xt = np.ascontiguousarray(x.T)

    in_maps = []
    for c in range(NCORES):
        s = c * TLOC
        idx = (np.arange(s - H, s + TLOC)) % T
        m = {'xT': np.ascontiguousarray(xt[:, idx])}
        for k in WNAMES:
            m[k] = W[k]
        for k in BNAMES:
            m[k] = col(Bv[bmap[k]])
        for k, v in mixes.items():
            m[k] = col(v)
        m['onescol'] = np.ones((128, 1), f32)
        m['onesrow'] = np.ones((1, 128), f32)
        m['ew'] = col(ew)
        m['eu'] = col(eu)
        m['cmask'] = np.full((D, 1), 0.0 if c == 0 else 1.0, f32)
        in_maps.append(m)
    return in_maps


_CACHED = {}
TRACE = False
LAST = {}


def kernel(**inputs):
    if 'nc' not in _CACHED:
        _CACHED['nc'] = build_kernel()
    nc = _CACHED['nc']
    in_maps = prep_inputs(inputs)
    kw = {}
    if TRACE:
        kw = dict(trace=True, trace_cores=list(range(NCORES)))
    res = run_bass_kernel_spmd(nc, in_maps, list(range(NCORES)), **kw)
    LAST['res'] = res
    parts = [np.asarray(res.results[c]['outT']) for c in range(NCORES)]
    out = np.concatenate(parts, axis=1).T
    return np.ascontiguousarray(out.astype(np.float32))


if __name__ == '__main__':
    import reference
    inputs = {k: np.asarray(v) for k, v in reference.setup_inputs().items()}
    out = kernel(**inputs)
    print('out', out.shape, out.dtype)



# revision 14
# speedup vs baseline: 3.1131x; 3.1131x over previous
"""RWKV block (T=8192, D=2048) on 8 Trainium2 NeuronCores — v2.

Data-parallel over the sequence (1024 tokens/core) with a 64-token
recomputed warmup prefix (power-decay attention forgets at e^{-|w|},
|w| >~ 0.6, so 64 steps reproduce the WKV state to ~e^{-40}).
Feature-major layout [D partitions, tokens free]; token shift is a
free-axis offset; LayerNorm stats are ones-matmuls on the PE; the WKV
recurrence is two tensor_tensor_scan linear scans per channel tile.

v2 vs the 3.42 ms baseline:
- bf16 weights + bf16 GEMM activations (1 cyc/row on the PE, half the
  HBM weight bytes, 1 KiB DMA descriptors).
- Blocks [64 warmup | 512 | 512]; the warmup shares the first main
  block's weight stream (one weight pass per GEMM call, two PSUM column
  groups per panel) -> 14 full weight passes total (~118 MB).
- Weight panels host-relaid [ktile, panel, 128, 512] contiguous; panel
  DMAs split across both HWDGE issue queues (sync + scalar).
- GEMM order k,v,r: the WKV scans (DVE) run under the r-GEMM; sigmoid
  is fused into the wkv*r multiply in the r-GEMM callback.
- Deep software pipelining: block B's k/v/r GEMMs are emitted between
  block A's r-GEMM and atto; LN2/fmix DVE work hides under the next
  GEMM. rz spills to DRAM and reloads for the final output add.
- SBUF block tensors live in 6 manually-scheduled slot families (tag
  reuse in emission order) to fit the bf16 512-col working set.
"""
import sys
if '/opt/trn_rl_repo' not in sys.path:
    sys.path.insert(0, '/opt/trn_rl_repo')

from contextlib import ExitStack
import numpy as np
import ml_dtypes

import concourse.bass as bass
import concourse.tile as tile
from concourse import bacc, mybir
from concourse.bass import _add_dep_helper
from concourse.bass_utils import run_bass_kernel_spmd

F32 = mybir.dt.float32
BF16 = mybir.dt.bfloat16
AF = mybir.ActivationFunctionType
OP = mybir.AluOpType

D = 2048
T = 8192
NCORES = 8
TLOC = T // NCORES          # 1024 main tokens per core
H = 64                      # warmup tokens
TBUF = H + TLOC             # 1088
NT = D // 128               # 16 partition tiles
NPAN = 4                    # weight panel iters per GEMM
PW = 512                    # panel width (j-cols per panel load)
BS = 512                    # main block size
W0, WN = 0, H
A0 = H
B0 = H + BS

WNAMES = ['wk', 'wv', 'wr', 'wo', 'wfk', 'wfv', 'wfr']
BNAMES = ['bk', 'bv', 'br', 'bo', 'bfk', 'bfv', 'bfr']
VNAMES = ['mixk', 'mixv', 'mixr', 'fmixk', 'fmixr', 'ew', 'eu', 'cmask']


def build_kernel():
    nc = bacc.Bacc()
    xbT = nc.declare_dram_parameter('xbT', [D, TBUF], BF16, isOutput=False)
    onescol = nc.declare_dram_parameter('onescol', [128, 1], BF16, isOutput=False)
    onesrow = nc.declare_dram_parameter('onesrow', [1, 128], BF16, isOutput=False)
    wd = {n: nc.declare_dram_parameter(n, [NT, NPAN, 128, PW], BF16,
                                       isOutput=False) for n in WNAMES}
    vd = {n: nc.declare_dram_parameter(n, [D, 1], F32, isOutput=False)
          for n in BNAMES + VNAMES}
    rzT = nc.declare_dram_parameter('rzT', [D, 2 * BS], BF16, isOutput=True)
    outT = nc.declare_dram_parameter('outT', [D, TLOC], BF16, isOutput=True)

    xbTt = xbT.rearrange('(n p) t -> n p t', p=128)
    rzTt = rzT.rearrange('(n p) t -> n p t', p=128)
    outTt = outT.rearrange('(n p) t -> n p t', p=128)
    vdt = {n: v.rearrange('(n p) o -> p (n o)', p=128) for n, v in vd.items()}

    with tile.TileContext(nc) as tc:
        with ExitStack() as ctx:
            kern(ctx, tc, xbTt, wd, vdt, rzTt, outTt, onescol, onesrow)
    nc.compile()
    return nc


def kern(ctx, tc, xbTt, wd, vdt, rzTt, outTt, onescol, onesrow):
    nc = tc.nc

    cons = ctx.enter_context(tc.tile_pool(name='cons', bufs=1))
    cv = {}
    for n in BNAMES + VNAMES:
        cvt = cons.tile([128, NT], F32, tag=f'cv_{n}', name=f'cv_{n}')
        nc.gpsimd.dma_start(cvt[:], vdt[n])
        cv[n] = [cvt[:, i:i + 1] for i in range(NT)]
    ones = cons.tile([128, 1], BF16, tag='ones', name='ones')
    nc.gpsimd.dma_start(ones[:], onescol[:])
    ones_row = cons.tile([1, 128], BF16, tag='ones_row', name='ones_row')
    nc.gpsimd.dma_start(ones_row[:], onesrow[:])

    colp = ctx.enter_context(tc.tile_pool(name='colp', bufs=1))
    Ucol = [colp.tile([128, 1], BF16, tag=f'uc{i}', name=f'uc{i}') for i in range(NT)]
    U2col = [colp.tile([128, 1], BF16, tag=f'u2c{i}', name=f'u2c{i}') for i in range(NT)]
    Acol = [colp.tile([128, 1], F32, tag=f'acl{i}', name=f'acl{i}') for i in range(NT)]
    Bcol = [colp.tile([128, 1], F32, tag=f'bcl{i}', name=f'bcl{i}') for i in range(NT)]
    for i in range(NT):
        nc.vector.memset(Ucol[i][:], 0.0)
        nc.vector.memset(U2col[i][:], 0.0)
        nc.vector.memset(Acol[i][:], 0.0)
        nc.vector.memset(Bcol[i][:], 0.0)

    wpool = ctx.enter_context(tc.tile_pool(name='wpool', bufs=2))
    fam = ctx.enter_context(tc.tile_pool(name='fam', bufs=1))
    wsm = ctx.enter_context(tc.tile_pool(name='wsm', bufs=1))
    upool = ctx.enter_context(tc.tile_pool(name='upool', bufs=1))
    rzp = ctx.enter_context(tc.tile_pool(name='rzp', bufs=17))
    scr = ctx.enter_context(tc.tile_pool(name='scr', bufs=1))
    rows = ctx.enter_context(tc.tile_pool(name='rows', bufs=1))
    psg = ctx.enter_context(tc.tile_pool(name='psg', bufs=4, space='PSUM'))
    pss = ctx.enter_context(tc.tile_pool(name='pss', bufs=2, space='PSUM'))

    def F(famid, i, n, nm):
        return fam.tile([128, n], BF16, tag=f'f{famid}_{i}', name=nm)

    def Wt(nm):
        return wsm.tile([128, WN], BF16, tag='wrot', name=nm, bufs=48)

    def pe_guard(aps):
        """Fused-LDW matmuls can carry only ONE sync wait in the ISA. Emit a
        PE NoOp that *reads* the given APs: Tile assigns the cross-engine
        waits to it, and matmuls ordered behind it on the PE queue inherit
        the observed clocks (waits elided)."""
        eng = nc.tensor
        inst = mybir.InstNoOp(
            name=nc.get_next_instruction_name(),
            text_hint='pe_guard', bass_nofuse=True,
            ins=[eng.lower_ap(a) for a in aps])
        return eng.add_instruction(inst)

    def gemm(wname, chunks):
        """chunks: list of (rhs16, ncols, cb). One weight pass; per panel
        iter, one PSUM accumulation group per (jtile, chunk)."""
        allr = [r for rhs, _, _ in chunks for r in rhs]
        for p in range(NPAN):
            panels = []
            for kt in range(NT):
                pt = wpool.tile([128, PW], BF16, tag=f'w{kt}', name=f'w{kt}')
                eng = nc.sync if kt % 2 == 0 else nc.scalar
                eng.dma_start(pt[:], wd[wname][kt, p])
                panels.append(pt)
            guard = pe_guard([p_[:] for p_ in panels] + allr)
            for jj in range(4):
                jt = p * 4 + jj
                for rhs, ncols, cb in chunks:
                    ps = psg.tile([128, ncols], F32, tag='ps', name='ps',
                                  padded_shape=[128, BS])
                    for kt in range(NT):
                        mm = nc.tensor.matmul(
                            ps[:], panels[kt][:, jj * 128:(jj + 1) * 128],
                            rhs[kt], start=(kt == 0), stop=(kt == NT - 1))
                        _add_dep_helper(mm.ins, guard.ins, sync=False,
                                        reason='order after guard')
                    cb(jt, ps[:])

    def ln_stats(xslices, n):
        """Per-token mean/rstd over partitions via ones-matmuls.
        xslices: 16 bf16 APs [128, n]. Returns (s_b, ms_b) [128, n] PSUM."""
        ps_s = pss.tile([1, n], F32, tag='st0', name='st0', padded_shape=[1, BS])
        ps_q = pss.tile([1, n], F32, tag='st1', name='st1', padded_shape=[1, BS])
        sq0 = scr.tile([128, n], BF16, tag='sq', name='sq', bufs=1,
                       padded_shape=[128, BS])
        nc.scalar.activation(sq0[:], xslices[0], AF.Square)
        guard = pe_guard(list(xslices) + [sq0[:], ones[:]])
        for kt in range(NT):
            if kt == 0:
                sq = sq0
            else:
                sq = scr.tile([128, n], BF16, tag='sq', name='sq', bufs=1,
                              padded_shape=[128, BS])
                nc.scalar.activation(sq[:], xslices[kt], AF.Square)
            mm = nc.tensor.matmul(ps_s[:], ones[:], xslices[kt],
                                  start=(kt == 0), stop=(kt == NT - 1))
            _add_dep_helper(mm.ins, guard.ins, sync=False, reason='g')
            mm2 = nc.tensor.matmul(ps_q[:], ones[:], sq[:],
                                   start=(kt == 0), stop=(kt == NT - 1))
            _add_dep_helper(mm2.ins, guard.ins, sync=False, reason='g')
        mean = rows.tile([1, n], BF16, tag='mean', name='mean', padded_shape=[1, BS])
        var = rows.tile([1, n], BF16, tag='var', name='var', padded_shape=[1, BS])
        m2 = rows.tile([1, n], BF16, tag='tmp', name='m2', padded_shape=[1, BS])
        nc.vector.tensor_scalar_mul(mean[:], ps_s[:], 1.0 / D)
        nc.vector.tensor_scalar_mul(var[:], ps_q[:], 1.0 / D)
        nc.vector.tensor_mul(m2[:], mean[:], mean[:])
        nc.vector.tensor_sub(var[:], var[:], m2[:])
        nc.vector.tensor_scalar_add(var[:], var[:], 1e-5)
        lnv = rows.tile([1, n], BF16, tag='tmp', name='lnv', padded_shape=[1, BS])
        nc.scalar.activation(lnv[:], var[:], AF.Ln)
        rstd = rows.tile([1, n], BF16, tag='rstd', name='rstd', padded_shape=[1, BS])
        nc.scalar.activation(rstd[:], lnv[:], AF.Exp, scale=-0.5)
        ms = rows.tile([1, n], BF16, tag='ms', name='ms', padded_shape=[1, BS])
        nc.vector.tensor_mul(ms[:], mean[:], rstd[:])
        s_b = pss.tile([128, n], F32, tag='st0', name='s_b', padded_shape=[128, BS])
        ms_b = pss.tile([128, n], F32, tag='st1', name='ms_b', padded_shape=[128, BS])
        guard2 = pe_guard([rstd[:], ms[:], ones_row[:]])
        mmb = nc.tensor.matmul(s_b[:], ones_row[:], rstd[:], start=True, stop=True)
        _add_dep_helper(mmb.ins, guard2.ins, sync=False, reason='g2')
        mmb2 = nc.tensor.matmul(ms_b[:], ones_row[:], ms[:], start=True, stop=True)
        _add_dep_helper(mmb2.ins, guard2.ins, sync=False, reason='g2')
        return s_b, ms_b

    def build_U(Utiles, xtiles, s_b, ms_b, ucols, n):
        for i in range(NT):
            nc.vector.tensor_copy(Utiles[i][:, 0:1], ucols[i][:])
            t1 = scr.tile([128, n], BF16, tag='t1', name='t1', bufs=1,
                          padded_shape=[128, BS])
            nc.vector.tensor_mul(t1[:], xtiles[i], s_b[:])
            nc.vector.tensor_sub(Utiles[i][:, 1:n + 1], t1[:], ms_b[:])
            nc.vector.tensor_copy(ucols[i][:], Utiles[i][:, n:n + 1])

    def build_mix(Utiles, mixname, out_tiles, n):
        for i in range(NT):
            dt_ = scr.tile([128, n], BF16, tag='dtmp', name='dtmp', bufs=1,
                           padded_shape=[128, BS])
            nc.vector.tensor_sub(dt_[:], Utiles[i][:, 1:n + 1], Utiles[i][:, 0:n])
            nc.vector.scalar_tensor_tensor(
                out_tiles[i][:], dt_[:], cv[mixname][i],
                Utiles[i][:, 0:n], OP.mult, OP.add)

    def scans(EKt, vvt, wkvt, n, mask_carry):
        """Per i-tile: EKV=EK*vv; A/B linear scans with carried state;
        wkv = (eu*EKV + A_prev) / (eu*EK + B_prev)."""
        for i in range(NT):
            ekv = scr.tile([128, n], BF16, tag='ekv', name='ekv', bufs=1,
                           padded_shape=[128, BS])
            nc.vector.tensor_mul(ekv[:], EKt[i][:], vvt[i][:])
            Aw = scr.tile([128, n + 1], BF16, tag='Aw', name='Aw', bufs=1,
                          padded_shape=[128, BS + 1])
            Bw = scr.tile([128, n + 1], BF16, tag='Bw', name='Bw', bufs=1,
                          padded_shape=[128, BS + 1])
            if mask_carry:
                nc.vector.tensor_mul(Aw[:, 0:1], Acol[i][:], cv['cmask'][i])
                nc.vector.tensor_mul(Bw[:, 0:1], Bcol[i][:], cv['cmask'][i])
            else:
                nc.vector.tensor_copy(Aw[:, 0:1], Acol[i][:])
                nc.vector.tensor_copy(Bw[:, 0:1], Bcol[i][:])
            ewb = cv['ew'][i].broadcast_to([128, n])
            nc.vector.tensor_tensor_scan(Aw[:, 1:n + 1], ewb, ekv[:],
                                         Aw[:, 0:1], OP.mult, OP.add)
            nc.vector.tensor_tensor_scan(Bw[:, 1:n + 1], ewb, EKt[i][:],
                                         Bw[:, 0:1], OP.mult, OP.add)
            nc.vector.tensor_copy(Acol[i][:], Aw[:, n:n + 1])
            nc.vector.tensor_copy(Bcol[i][:], Bw[:, n:n + 1])
            num = scr.tile([128, n], BF16, tag='num', name='num', bufs=1,
                           padded_shape=[128, BS])
            den = scr.tile([128, n], F32, tag='den', name='den', bufs=1,
                           padded_shape=[128, BS])
            nc.vector.scalar_tensor_tensor(num[:], ekv[:], cv['eu'][i],
                                           Aw[:, 0:n], OP.mult, OP.add)
            nc.vector.scalar_tensor_tensor(den[:], EKt[i][:], cv['eu'][i],
                                           Bw[:, 0:n], OP.mult, OP.add)
            rec = scr.tile([128, n], F32, tag='rec', name='rec', bufs=1,
                           padded_shape=[128, BS])
            nc.vector.reciprocal_approx_fast(rec[:], den[:])
            nc.vector.tensor_mul(wkvt[i][:], num[:], rec[:])

    # ---------------- emission ----------------
    xbW = [wsm.tile([128, WN], BF16, tag='xw', name=f'xw{i}', bufs=16)
           for i in range(NT)]
    xbA = [F(0, i, BS, f'xa{i}') for i in range(NT)]
    xbB = [F(2, i, BS, f'xb{i}') for i in range(NT)]
    for i in range(NT):
        nc.gpsimd.dma_start(xbW[i][:], xbTt[i, :, W0:W0 + WN])
        nc.gpsimd.dma_start(xbA[i][:], xbTt[i, :, A0:A0 + BS])
        nc.gpsimd.dma_start(xbB[i][:], xbTt[i, :, B0:B0 + BS])

    # LN1 + U + mixes, warmup
    sbW, msW = ln_stats([xbW[i][:] for i in range(NT)], WN)
    UW = [upool.tile([128, WN + 1], BF16, tag=f'u{i}', name=f'uw{i}',
                     padded_shape=[128, BS + 1]) for i in range(NT)]
    build_U(UW, [xbW[i][:] for i in range(NT)], sbW, msW, Ucol, WN)
    inkW = [Wt(f'ikw{i}') for i in range(NT)]
    invW = [Wt(f'ivw{i}') for i in range(NT)]
    inrW = [Wt(f'irw{i}') for i in range(NT)]
    build_mix(UW, 'mixk', inkW, WN)
    build_mix(UW, 'mixv', invW, WN)
    build_mix(UW, 'mixr', inrW, WN)

    # LN1 + U + mixk, block A
    sbA, msA = ln_stats([xbA[i][:] for i in range(NT)], BS)
    UA = [upool.tile([128, BS + 1], BF16, tag=f'u{i}', name=f'ua{i}') for i in range(NT)]
    build_U(UA, [xbA[i][:] for i in range(NT)], sbA, msA, Ucol, BS)
    inkA = [F(3, i, BS, f'ika{i}') for i in range(NT)]
    build_mix(UA, 'mixk', inkA, BS)

    # wk over W+A
    EKW = [wsm.tile([128, WN], BF16, tag='ekwd', name=f'ekw{i}', bufs=16)
           for i in range(NT)]
    EKA = [F(1, i, BS, f'eka{i}') for i in range(NT)]

    def mk_cb_act(tiles, func, bias):
        def cb(jt, ps):
            nc.scalar.activation(tiles[jt][:], ps, func, bias=cv[bias][jt])
        return cb
    gemm('wk', [([t[:] for t in inkW], WN, mk_cb_act(EKW, AF.Exp, 'bk')),
                ([t[:] for t in inkA], BS, mk_cb_act(EKA, AF.Exp, 'bk'))])

    # mixes v, r for A (DVE, during wk)
    invA = [F(0, i, BS, f'iva{i}') for i in range(NT)]
    inrA = [F(4, i, BS, f'ira{i}') for i in range(NT)]
    build_mix(UA, 'mixv', invA, BS)
    build_mix(UA, 'mixr', inrA, BS)

    # wv over W+A
    vvW = [wsm.tile([128, WN], BF16, tag='vvwd', name=f'vvw{i}', bufs=16)
           for i in range(NT)]
    vvA = [F(3, i, BS, f'vva{i}') for i in range(NT)]
    gemm('wv', [([t[:] for t in invW], WN, mk_cb_act(vvW, AF.Identity, 'bv')),
                ([t[:] for t in invA], BS, mk_cb_act(vvA, AF.Identity, 'bv'))])

    # scans W then A (DVE; run under wr)
    wkvW = [Wt(f'wvw{i}') for i in range(NT)]
    wkvA = [F(0, i, BS, f'wva{i}') for i in range(NT)]
    scans(EKW, vvW, wkvW, WN, mask_carry=False)
    scans(EKA, vvA, wkvA, BS, mask_carry=True)  # core0 zeroes carry at W->A

    # wr over W+A; sigmoid fused into wkv*r
    wkvrW = [wsm.tile([128, WN], BF16, tag='wrwd', name=f'wrw{i}', bufs=16)
           for i in range(NT)]
    wkvrA = [F(5, i, BS, f'wra{i}') for i in range(NT)]

    def mk_cb_wr(wkvr, wkv, n, tag):
        def cb(jt, ps):
            rs = scr.tile([128, n], BF16, tag=tag, name=tag, bufs=1,
                          padded_shape=[128, BS])
            nc.scalar.activation(rs[:], ps, AF.Sigmoid, bias=cv['br'][jt])
            nc.vector.tensor_mul(wkvr[jt][:], wkv[jt][:], rs[:])
        return cb
    gemm('wr', [([t[:] for t in inrW], WN, mk_cb_wr(wkvrW, wkvW, WN, 'rsw')),
                ([t[:] for t in inrA], BS, mk_cb_wr(wkvrA, wkvA, BS, 'rsa'))])

    # LN1 + U + mixes for B (DVE, during wr/wk(B))
    sbB, msB = ln_stats([xbB[i][:] for i in range(NT)], BS)
    UB = [upool.tile([128, BS + 1], BF16, tag=f'u{i}', name=f'ub{i}') for i in range(NT)]
    build_U(UB, [xbB[i][:] for i in range(NT)], sbB, msB, Ucol, BS)
    inkB = [F(2, i, BS, f'ikb{i}') for i in range(NT)]
    build_mix(UB, 'mixk', inkB, BS)
    invB = [F(0, i, BS, f'ivb{i}') for i in range(NT)]
    inrB = [F(4, i, BS, f'irb{i}') for i in range(NT)]
    build_mix(UB, 'mixv', invB, BS)
    build_mix(UB, 'mixr', inrB, BS)

    # k/v/r over B
    EKB = [F(1, i, BS, f'ekb{i}') for i in range(NT)]
    gemm('wk', [([t[:] for t in inkB], BS, mk_cb_act(EKB, AF.Exp, 'bk'))])
    vvB = [F(3, i, BS, f'vvb{i}') for i in range(NT)]
    gemm('wv', [([t[:] for t in invB], BS, mk_cb_act(vvB, AF.Identity, 'bv'))])
    wkvB = [F(0, i, BS, f'wvb{i}') for i in range(NT)]
    scans(EKB, vvB, wkvB, BS, mask_carry=False)
    wkvrB = [F(3, i, BS, f'wrb{i}') for i in range(NT)]
    gemm('wr', [([t[:] for t in inrB], BS, mk_cb_wr(wkvrB, wkvB, BS, 'rsa'))])

    # atto over W+A; rz = psum + bo + x; main-block rz spills to DRAM
    rzW = [wsm.tile([128, WN], BF16, tag='rzwd', name=f'rzw{i}', bufs=16)
           for i in range(NT)]
    rzA = [None] * NT
    rzB = [None] * NT

    class Roller:
        """Rolling-prefetch DRAM->SBUF loader: tile jt+DEPTH is allocated
        and its DMA issued at consume(jt), keeping FIFO slot reuse legal
        at small bufs while hiding the DMA under the GEMM panels."""
        DEPTH = 3

        def __init__(self, src_t0):
            self.src_t0 = src_t0
            self.tiles = {}
            for jt in range(self.DEPTH):
                self._fetch(jt)

        def _fetch(self, jt):
            if jt < NT:
                tl = scr.tile([128, BS], BF16, tag='roll', name='roll',
                              bufs=self.DEPTH + 2)
                nc.gpsimd.dma_start(tl[:], self.src(jt))
                self.tiles[jt] = tl

        def src(self, jt):
            return xbTt[jt, :, self.src_t0:self.src_t0 + BS]

        def consume(self, jt):
            self._fetch(jt + self.DEPTH)
            return self.tiles.pop(jt)

    class RzRoller(Roller):
        def __init__(self, rzoff):
            self.rzoff = rzoff
            super().__init__(0)

        def src(self, jt):
            return rzTt[jt, :, self.rzoff:self.rzoff + BS]

    def mk_cb_rzw():
        def cb(jt, ps):
            nc.vector.scalar_tensor_tensor(rzW[jt][:], ps, cv['bo'][jt],
                                           xbW[jt][:], OP.add, OP.add)
        return cb

    def mk_cb_rz(rzlist, x2roll, rzoff):
        def cb(jt, ps):
            rz = rzp.tile([128, BS], BF16, tag='rz', name='rz')
            rzlist[jt] = rz
            x2 = x2roll.consume(jt)
            nc.vector.scalar_tensor_tensor(rz[:], ps, cv['bo'][jt],
                                           x2[:], OP.add, OP.add)
            eng = nc.sync if jt % 2 == 0 else nc.scalar
            eng.dma_start(rzTt[jt, :, rzoff:rzoff + BS], rz[:])
        return cb

    x2A = Roller(A0)
    gemm('wo', [([t[:] for t in wkvrW], WN, mk_cb_rzw()),
                ([t[:] for t in wkvrA], BS, mk_cb_rz(rzA, x2A, 0))])

    # LN2 + U2 for W and A
    sb2W, ms2W = ln_stats([rzW[i][:] for i in range(NT)], WN)
    U2W = [upool.tile([128, WN + 1], BF16, tag=f'u{i}', name=f'u2w{i}',
                      padded_shape=[128, BS + 1]) for i in range(NT)]
    build_U(U2W, [rzW[i][:] for i in range(NT)], sb2W, ms2W, U2col, WN)
    sb2A, ms2A = ln_stats([rzA[i][:] for i in range(NT)], BS)
    U2A = [upool.tile([128, BS + 1], BF16, tag=f'u{i}', name=f'u2a{i}') for i in range(NT)]
    build_U(U2A, [rzA[i][:] for i in range(NT)], sb2A, ms2A, U2col, BS)

    # atto over B
    x2B = Roller(B0)
    gemm('wo', [([t[:] for t in wkvrB], BS, mk_cb_rz(rzB, x2B, BS))])

    # FFN mixes A (before U2B alloc: U2A readers must be emitted first)
    fmkA = [F(0, i, BS, f'fka{i}') for i in range(NT)]
    fmrA = [F(3, i, BS, f'fra{i}') for i in range(NT)]
    build_mix(U2A, 'fmixk', fmkA, BS)
    build_mix(U2A, 'fmixr', fmrA, BS)

    # LN2 + U2 for B
    sb2B, ms2B = ln_stats([rzB[i][:] for i in range(NT)], BS)
    U2B = [upool.tile([128, BS + 1], BF16, tag=f'u{i}', name=f'u2b{i}') for i in range(NT)]
    build_U(U2B, [rzB[i][:] for i in range(NT)], sb2B, ms2B, U2col, BS)

    # FFN block A
    kf2A = [F(1, i, BS, f'k2a{i}') for i in range(NT)]

    def mk_cb_kf(kf2):
        def cb(jt, ps):
            kf = scr.tile([128, BS], BF16, tag='kf', name='kf', bufs=1)
            nc.scalar.activation(kf[:], ps, AF.Identity, bias=cv['bfk'][jt])
            nc.vector.scalar_tensor_tensor(kf2[jt][:], kf[:], 0.0, kf[:],
                                           OP.max, OP.mult)
        return cb
    gemm('wfk', [([t[:] for t in fmkA], BS, mk_cb_kf(kf2A))])

    rfA = [F(5, i, BS, f'rfa{i}') for i in range(NT)]
    gemm('wfr', [([t[:] for t in fmrA], BS, mk_cb_act(rfA, AF.Sigmoid, 'bfr'))])

    # FFN mixes B (during wfk/wfr A)
    fmkB = [F(2, i, BS, f'fkb{i}') for i in range(NT)]
    fmrB = [F(4, i, BS, f'frb{i}') for i in range(NT)]
    build_mix(U2B, 'fmixk', fmkB, BS)
    build_mix(U2B, 'fmixr', fmrB, BS)

    def mk_cb_out(rzroll, rflist, t0):
        def cb(jt, ps):
            t3 = scr.tile([128, BS], BF16, tag='t3o', name='t3o', bufs=1)
            nc.vector.scalar_tensor_tensor(t3[:], ps, cv['bfv'][jt],
                                           rflist[jt][:], OP.add, OP.mult)
            rzl = rzroll.consume(jt)
            ot = scr.tile([128, BS], BF16, tag='ot', name='ot', bufs=2)
            nc.vector.tensor_add(ot[:], t3[:], rzl[:])
            eng = nc.sync if jt % 2 == 0 else nc.scalar
            eng.dma_start(outTt[jt, :, t0:t0 + BS], ot[:])
        return cb
    rzlA = RzRoller(0)
    gemm('wfv', [([t[:] for t in kf2A], BS, mk_cb_out(rzlA, rfA, 0))])

    # FFN block B
    kf2B = [F(0, i, BS, f'k2b{i}') for i in range(NT)]
    gemm('wfk', [([t[:] for t in fmkB], BS, mk_cb_kf(kf2B))])
    rfB = [F(1, i, BS, f'rfb{i}') for i in range(NT)]
    gemm('wfr', [([t[:] for t in fmrB], BS, mk_cb_act(rfB, AF.Sigmoid, 'bfr'))])
    rzlB = RzRoller(BS)
    gemm('wfv', [([t[:] for t in kf2B], BS, mk_cb_out(rzlB, rfB, BS))])


def prep_inputs(inputs):
    f32 = np.float32
    bf16 = ml_dtypes.bfloat16
    x = np.asarray(inputs['x'], f32)
    g1, b1 = np.asarray(inputs['ln1_g'], f32), np.asarray(inputs['ln1_b'], f32)
    g2, b2 = np.asarray(inputs['ln2_g'], f32), np.asarray(inputs['ln2_b'], f32)
    W, Bv = {}, {}
    for key, nm, g, b in [('wk', 'attk', g1, b1), ('wv', 'attv', g1, b1),
                          ('wr', 'attr', g1, b1), ('wfk', 'ffnk', g2, b2),
                          ('wfr', 'ffnr', g2, b2)]:
        w = np.asarray(inputs[nm + '_w'], f32)
        W[key] = (w * g[None, :]).T
        Bv[key] = (np.asarray(inputs[nm + '_b'], f32) + w @ b).astype(f32)
    for key, nm in [('wo', 'atto'), ('wfv', 'ffnv')]:
        w = np.asarray(inputs[nm + '_w'], f32)
        W[key] = w.T
        Bv[key] = np.asarray(inputs[nm + '_b'], f32)
    Wp = {}
    for key, w in W.items():
        wp = w.reshape(NT, 128, NPAN, PW).transpose(0, 2, 1, 3)
        Wp[key] = np.ascontiguousarray(wp.astype(bf16))
    bmap = dict(zip(BNAMES, ['wk', 'wv', 'wr', 'wo', 'wfk', 'wfv', 'wfr']))
    col = lambda a: np.ascontiguousarray(np.asarray(a, f32).reshape(D, 1))
    mixes = {'mixk': inputs['attmixk'], 'mixv': inputs['attmixv'],
             'mixr': inputs['attmixr'], 'fmixk': inputs['ffnmixk'],
             'fmixr': inputs['ffnmixr']}
    ew = np.exp(-np.exp(np.asarray(inputs['time_decay'], f32))).astype(f32)
    eu = np.exp(np.asarray(inputs['time_first'], f32)).astype(f32)
    xt = np.ascontiguousarray(x.T)

    in_maps = []
    for c in range(NCORES):
        s = c * TLOC
        idx = (np.arange(s - H, s + TLOC)) % T
        m = {'xbT': np.ascontiguousarray(xt[:, idx].astype(bf16))}
        for k in WNAMES:
            m[k] = Wp[k]
        for k in BNAMES:
            m[k] = col(Bv[bmap[k]])
        for k, v in mixes.items():
            m[k] = col(v)
        m['onescol'] = np.ones((128, 1), bf16)
        m['onesrow'] = np.ones((1, 128), bf16)
        m['ew'] = col(ew)
        m['eu'] = col(eu)
        m['cmask'] = np.full((D, 1), 0.0 if c == 0 else 1.0, f32)
        in_maps.append(m)
    return in_maps


_CACHED = {}
TRACE = False
LAST = {}


def kernel(**inputs):
    if 'nc' not in _CACHED:
        _CACHED['nc'] = build_kernel()
    nc = _CACHED['nc']
    in_maps = prep_inputs(inputs)
    kw = {}
    if TRACE:
        kw = dict(trace=True, trace_cores=list(range(NCORES)))
    res = run_bass_kernel_spmd(nc, in_maps, list(range(NCORES)), **kw)
    LAST['res'] = res
    parts = [np.asarray(res.results[c]['outT']) for c in range(NCORES)]
    out = np.concatenate(parts, axis=1).T
    return np.ascontiguousarray(out.astype(np.float32))


if __name__ == '__main__':
    import reference
    inputs = {k: np.asarray(v) for k, v in reference.setup_inputs().items()}
    out = kernel(**inputs)
    print('out', out.shape, out.dtype)


# revision 15
# speedup vs baseline: 3.2005x; 1.0281x over previous
"""RWKV block (T=8192, D=2048) on 8 Trainium2 NeuronCores — v2.

Data-parallel over the sequence (1024 tokens/core) with a 64-token
recomputed warmup prefix (power-decay attention forgets at e^{-|w|},
|w| >~ 0.6, so 32 steps reproduce the WKV state to ~e^{-18}).
Feature-major layout [D partitions, tokens free]; token shift is a
free-axis offset; LayerNorm stats are ones-matmuls on the PE; the WKV
recurrence is two tensor_tensor_scan linear scans per channel tile.

v2 vs the 3.42 ms baseline:
- bf16 weights + bf16 GEMM activations (1 cyc/row on the PE, half the
  HBM weight bytes, 1 KiB DMA descriptors).
- Blocks [64 warmup | 512 | 512]; the warmup shares the first main
  block's weight stream (one weight pass per GEMM call, two PSUM column
  groups per panel) -> 14 full weight passes total (~118 MB).
- Weight panels host-relaid [ktile, panel, 128, 512] contiguous; panel
  DMAs split across both HWDGE issue queues (sync + scalar).
- GEMM order k,v,r: the WKV scans (DVE) run under the r-GEMM; sigmoid
  is fused into the wkv*r multiply in the r-GEMM callback.
- Deep software pipelining: block B's k/v/r GEMMs are emitted between
  block A's r-GEMM and atto; LN2/fmix DVE work hides under the next
  GEMM. rz spills to DRAM and reloads for the final output add.
- SBUF block tensors live in 6 manually-scheduled slot families (tag
  reuse in emission order) to fit the bf16 512-col working set.
"""
import sys
if '/opt/trn_rl_repo' not in sys.path:
    sys.path.insert(0, '/opt/trn_rl_repo')

from contextlib import ExitStack
import numpy as np
import ml_dtypes

import concourse.bass as bass
import concourse.tile as tile
from concourse import bacc, mybir
from concourse.bass import _add_dep_helper
from concourse.bass_utils import run_bass_kernel_spmd

F32 = mybir.dt.float32
BF16 = mybir.dt.bfloat16
AF = mybir.ActivationFunctionType
OP = mybir.AluOpType

D = 2048
T = 8192
NCORES = 8
TLOC = T // NCORES          # 1024 main tokens per core
H = 32                      # warmup tokens
TBUF = H + TLOC             # 1088
NT = D // 128               # 16 partition tiles
NPAN = 4                    # weight panel iters per GEMM
PW = 512                    # panel width (j-cols per panel load)
BS = 512                    # main block size
W0, WN = 0, H
A0 = H
B0 = H + BS

WNAMES = ['wk', 'wv', 'wr', 'wo', 'wfk', 'wfv', 'wfr']
BNAMES = ['bk', 'bv', 'br', 'bo', 'bfk', 'bfv', 'bfr']
VNAMES = ['mixk', 'mixv', 'mixr', 'fmixk', 'fmixr', 'ew', 'eu', 'cmask']


def build_kernel():
    nc = bacc.Bacc()
    xbT = nc.declare_dram_parameter('xbT', [D, TBUF], BF16, isOutput=False)
    onescol = nc.declare_dram_parameter('onescol', [128, 1], BF16, isOutput=False)
    onesrow = nc.declare_dram_parameter('onesrow', [1, 128], BF16, isOutput=False)
    wd = {n: nc.declare_dram_parameter(n, [NT, NPAN, 128, PW], BF16,
                                       isOutput=False) for n in WNAMES}
    vd = {n: nc.declare_dram_parameter(n, [D, 1], F32, isOutput=False)
          for n in BNAMES + VNAMES}
    rzT = nc.declare_dram_parameter('rzT', [D, 2 * BS], BF16, isOutput=True)
    outT = nc.declare_dram_parameter('outT', [D, TLOC], BF16, isOutput=True)

    xbTt = xbT.rearrange('(n p) t -> n p t', p=128)
    rzTt = rzT.rearrange('(n p) t -> n p t', p=128)
    outTt = outT.rearrange('(n p) t -> n p t', p=128)
    vdt = {n: v.rearrange('(n p) o -> p (n o)', p=128) for n, v in vd.items()}

    with tile.TileContext(nc) as tc:
        with ExitStack() as ctx:
            kern(ctx, tc, xbTt, wd, vdt, rzTt, outTt, onescol, onesrow)
    nc.compile()
    return nc


def kern(ctx, tc, xbTt, wd, vdt, rzTt, outTt, onescol, onesrow):
    nc = tc.nc

    cons = ctx.enter_context(tc.tile_pool(name='cons', bufs=1))
    cv = {}
    for n in BNAMES + VNAMES:
        cvt = cons.tile([128, NT], F32, tag=f'cv_{n}', name=f'cv_{n}')
        nc.gpsimd.dma_start(cvt[:], vdt[n])
        cv[n] = [cvt[:, i:i + 1] for i in range(NT)]
    ones = cons.tile([128, 1], BF16, tag='ones', name='ones')
    nc.sync.dma_start(ones[:], onescol[:])
    ones_row = cons.tile([1, 128], BF16, tag='ones_row', name='ones_row')
    nc.scalar.dma_start(ones_row[:], onesrow[:])

    colp = ctx.enter_context(tc.tile_pool(name='colp', bufs=1))
    Ucol = [colp.tile([128, 1], BF16, tag=f'uc{i}', name=f'uc{i}') for i in range(NT)]
    U2col = [colp.tile([128, 1], BF16, tag=f'u2c{i}', name=f'u2c{i}') for i in range(NT)]
    Acol = [colp.tile([128, 1], F32, tag=f'acl{i}', name=f'acl{i}') for i in range(NT)]
    Bcol = [colp.tile([128, 1], F32, tag=f'bcl{i}', name=f'bcl{i}') for i in range(NT)]
    for i in range(NT):
        nc.vector.memset(Ucol[i][:], 0.0)
        nc.vector.memset(U2col[i][:], 0.0)
        nc.vector.memset(Acol[i][:], 0.0)
        nc.vector.memset(Bcol[i][:], 0.0)

    wpool = ctx.enter_context(tc.tile_pool(name='wpool', bufs=2))
    fam = ctx.enter_context(tc.tile_pool(name='fam', bufs=1))
    wsm = ctx.enter_context(tc.tile_pool(name='wsm', bufs=1))
    upool = ctx.enter_context(tc.tile_pool(name='upool', bufs=1))
    rzp = ctx.enter_context(tc.tile_pool(name='rzp', bufs=17))
    scr = ctx.enter_context(tc.tile_pool(name='scr', bufs=1))
    rows = ctx.enter_context(tc.tile_pool(name='rows', bufs=1))
    psg = ctx.enter_context(tc.tile_pool(name='psg', bufs=4, space='PSUM'))
    pss = ctx.enter_context(tc.tile_pool(name='pss', bufs=2, space='PSUM'))

    def F(famid, i, n, nm):
        return fam.tile([128, n], BF16, tag=f'f{famid}_{i}', name=nm)

    def Wt(nm):
        return wsm.tile([128, WN], BF16, tag='wrot', name=nm, bufs=48)

    def pe_guard(aps):
        """Fused-LDW matmuls can carry only ONE sync wait in the ISA. Emit a
        PE NoOp that *reads* the given APs: Tile assigns the cross-engine
        waits to it, and matmuls ordered behind it on the PE queue inherit
        the observed clocks (waits elided)."""
        eng = nc.tensor
        inst = mybir.InstNoOp(
            name=nc.get_next_instruction_name(),
            text_hint='pe_guard', bass_nofuse=True,
            ins=[eng.lower_ap(a) for a in aps])
        return eng.add_instruction(inst)

    def gemm(wname, chunks):
        """chunks: list of (rhs16, ncols, cb). One weight pass; per panel
        iter, one PSUM accumulation group per (jtile, chunk)."""
        for p in range(NPAN):
            panels = []
            for kt in range(NT):
                pt = wpool.tile([128, PW], BF16, tag=f'w{kt}', name=f'w{kt}')
                eng = nc.sync if kt % 2 == 0 else nc.scalar
                eng.dma_start(pt[:], wd[wname][kt, p])
                panels.append(pt)
            guards = [pe_guard([p_[:] for p_ in panels] + list(rhs))
                      for rhs, _, _ in chunks]
            for jj in range(4):
                jt = p * 4 + jj
                for ci, (rhs, ncols, cb) in enumerate(chunks):
                    ps = psg.tile([128, ncols], F32, tag='ps', name='ps',
                                  padded_shape=[128, BS])
                    for kt in range(NT):
                        mm = nc.tensor.matmul(
                            ps[:], panels[kt][:, jj * 128:(jj + 1) * 128],
                            rhs[kt], start=(kt == 0), stop=(kt == NT - 1))
                        _add_dep_helper(mm.ins, guards[ci].ins, sync=False,
                                        reason='order after guard')
                    cb(jt, ps[:])

    def ln_stats(xslices, n):
        """Per-token mean/rstd over partitions via ones-matmuls.
        xslices: 16 bf16 APs [128, n]. Returns (s_b, ms_b) [128, n] PSUM."""
        ps_s = pss.tile([1, n], F32, tag='st0', name='st0', padded_shape=[1, BS])
        ps_q = pss.tile([1, n], F32, tag='st1', name='st1', padded_shape=[1, BS])
        sq0 = scr.tile([128, n], BF16, tag='sq', name='sq', bufs=1,
                       padded_shape=[128, BS])
        nc.scalar.activation(sq0[:], xslices[0], AF.Square)
        guard = pe_guard(list(xslices) + [sq0[:], ones[:]])
        for kt in range(NT):
            if kt == 0:
                sq = sq0
            else:
                sq = scr.tile([128, n], BF16, tag='sq', name='sq', bufs=1,
                              padded_shape=[128, BS])
                nc.scalar.activation(sq[:], xslices[kt], AF.Square)
            mm = nc.tensor.matmul(ps_s[:], ones[:], xslices[kt],
                                  start=(kt == 0), stop=(kt == NT - 1))
            _add_dep_helper(mm.ins, guard.ins, sync=False, reason='g')
            mm2 = nc.tensor.matmul(ps_q[:], ones[:], sq[:],
                                   start=(kt == 0), stop=(kt == NT - 1))
            _add_dep_helper(mm2.ins, guard.ins, sync=False, reason='g')
        mean = rows.tile([1, n], BF16, tag='mean', name='mean', padded_shape=[1, BS])
        var = rows.tile([1, n], BF16, tag='var', name='var', padded_shape=[1, BS])
        m2 = rows.tile([1, n], BF16, tag='tmp', name='m2', padded_shape=[1, BS])
        nc.vector.tensor_scalar_mul(mean[:], ps_s[:], 1.0 / D)
        nc.vector.tensor_scalar_mul(var[:], ps_q[:], 1.0 / D)
        nc.vector.tensor_mul(m2[:], mean[:], mean[:])
        nc.vector.tensor_sub(var[:], var[:], m2[:])
        nc.vector.tensor_scalar_add(var[:], var[:], 1e-5)
        lnv = rows.tile([1, n], BF16, tag='tmp', name='lnv', padded_shape=[1, BS])
        nc.scalar.activation(lnv[:], var[:], AF.Ln)
        rstd = rows.tile([1, n], BF16, tag='rstd', name='rstd', padded_shape=[1, BS])
        nc.scalar.activation(rstd[:], lnv[:], AF.Exp, scale=-0.5)
        ms = rows.tile([1, n], BF16, tag='ms', name='ms', padded_shape=[1, BS])
        nc.vector.tensor_mul(ms[:], mean[:], rstd[:])
        s_b = pss.tile([128, n], F32, tag='st0', name='s_b', padded_shape=[128, BS])
        ms_b = pss.tile([128, n], F32, tag='st1', name='ms_b', padded_shape=[128, BS])
        guard2 = pe_guard([rstd[:], ms[:], ones_row[:]])
        mmb = nc.tensor.matmul(s_b[:], ones_row[:], rstd[:], start=True, stop=True)
        _add_dep_helper(mmb.ins, guard2.ins, sync=False, reason='g2')
        mmb2 = nc.tensor.matmul(ms_b[:], ones_row[:], ms[:], start=True, stop=True)
        _add_dep_helper(mmb2.ins, guard2.ins, sync=False, reason='g2')
        return s_b, ms_b

    def build_U(Utiles, xtiles, s_b, ms_b, ucols, n):
        for i in range(NT):
            nc.vector.tensor_copy(Utiles[i][:, 0:1], ucols[i][:])
            t1 = scr.tile([128, n], BF16, tag='t1', name='t1', bufs=1,
                          padded_shape=[128, BS])
            nc.vector.tensor_mul(t1[:], xtiles[i], s_b[:])
            nc.vector.tensor_sub(Utiles[i][:, 1:n + 1], t1[:], ms_b[:])
            nc.vector.tensor_copy(ucols[i][:], Utiles[i][:, n:n + 1])

    def build_mix(Utiles, mixname, out_tiles, n):
        for i in range(NT):
            dt_ = scr.tile([128, n], BF16, tag='dtmp', name='dtmp', bufs=1,
                           padded_shape=[128, BS])
            nc.vector.tensor_sub(dt_[:], Utiles[i][:, 1:n + 1], Utiles[i][:, 0:n])
            nc.vector.scalar_tensor_tensor(
                out_tiles[i][:], dt_[:], cv[mixname][i],
                Utiles[i][:, 0:n], OP.mult, OP.add)

    def scans(EKt, vvt, wkvt, n, mask_carry):
        """Per i-tile: EKV=EK*vv; A/B linear scans with carried state;
        wkv = (eu*EKV + A_prev) / (eu*EK + B_prev)."""
        for i in range(NT):
            ekv = scr.tile([128, n], BF16, tag='ekv', name='ekv', bufs=1,
                           padded_shape=[128, BS])
            nc.vector.tensor_mul(ekv[:], EKt[i][:], vvt[i][:])
            Aw = scr.tile([128, n + 1], BF16, tag='Aw', name='Aw', bufs=1,
                          padded_shape=[128, BS + 1])
            Bw = scr.tile([128, n + 1], BF16, tag='Bw', name='Bw', bufs=1,
                          padded_shape=[128, BS + 1])
            if mask_carry:
                nc.vector.tensor_mul(Aw[:, 0:1], Acol[i][:], cv['cmask'][i])
                nc.vector.tensor_mul(Bw[:, 0:1], Bcol[i][:], cv['cmask'][i])
            else:
                nc.vector.tensor_copy(Aw[:, 0:1], Acol[i][:])
                nc.vector.tensor_copy(Bw[:, 0:1], Bcol[i][:])
            ewb = cv['ew'][i].broadcast_to([128, n])
            nc.vector.tensor_tensor_scan(Aw[:, 1:n + 1], ewb, ekv[:],
                                         Aw[:, 0:1], OP.mult, OP.add)
            nc.vector.tensor_tensor_scan(Bw[:, 1:n + 1], ewb, EKt[i][:],
                                         Bw[:, 0:1], OP.mult, OP.add)
            nc.vector.tensor_copy(Acol[i][:], Aw[:, n:n + 1])
            nc.vector.tensor_copy(Bcol[i][:], Bw[:, n:n + 1])
            num = scr.tile([128, n], BF16, tag='num', name='num', bufs=1,
                           padded_shape=[128, BS])
            den = scr.tile([128, n], F32, tag='den', name='den', bufs=1,
                           padded_shape=[128, BS])
            nc.vector.scalar_tensor_tensor(num[:], ekv[:], cv['eu'][i],
                                           Aw[:, 0:n], OP.mult, OP.add)
            nc.vector.scalar_tensor_tensor(den[:], EKt[i][:], cv['eu'][i],
                                           Bw[:, 0:n], OP.mult, OP.add)
            rec = scr.tile([128, n], F32, tag='rec', name='rec', bufs=1,
                           padded_shape=[128, BS])
            nc.vector.reciprocal_approx_fast(rec[:], den[:])
            nc.vector.tensor_mul(wkvt[i][:], num[:], rec[:])

    # ---------------- emission ----------------
    xbW = [wsm.tile([128, WN], BF16, tag='xw', name=f'xw{i}', bufs=16)
           for i in range(NT)]
    xbA = [F(0, i, BS, f'xa{i}') for i in range(NT)]
    xbB = [F(2, i, BS, f'xb{i}') for i in range(NT)]
    for i in range(NT):
        eng = nc.sync if i % 2 == 0 else nc.scalar
        eng.dma_start(xbW[i][:], xbTt[i, :, W0:W0 + WN])
    for i in range(NT):
        eng = nc.sync if i % 2 == 0 else nc.scalar
        eng.dma_start(xbA[i][:], xbTt[i, :, A0:A0 + BS])
    for i in range(NT):
        eng = nc.sync if i % 2 == 0 else nc.scalar
        eng.dma_start(xbB[i][:], xbTt[i, :, B0:B0 + BS])

    # LN1 + U + mixes, warmup
    sbW, msW = ln_stats([xbW[i][:] for i in range(NT)], WN)
    UW = [upool.tile([128, WN + 1], BF16, tag=f'u{i}', name=f'uw{i}',
                     padded_shape=[128, BS + 1]) for i in range(NT)]
    build_U(UW, [xbW[i][:] for i in range(NT)], sbW, msW, Ucol, WN)
    inkW = [Wt(f'ikw{i}') for i in range(NT)]
    invW = [Wt(f'ivw{i}') for i in range(NT)]
    inrW = [Wt(f'irw{i}') for i in range(NT)]
    build_mix(UW, 'mixk', inkW, WN)
    build_mix(UW, 'mixv', invW, WN)
    build_mix(UW, 'mixr', inrW, WN)

    # LN1 + U + mixk, block A
    sbA, msA = ln_stats([xbA[i][:] for i in range(NT)], BS)
    UA = [upool.tile([128, BS + 1], BF16, tag=f'u{i}', name=f'ua{i}') for i in range(NT)]
    build_U(UA, [xbA[i][:] for i in range(NT)], sbA, msA, Ucol, BS)
    inkA = [F(3, i, BS, f'ika{i}') for i in range(NT)]
    build_mix(UA, 'mixk', inkA, BS)

    # wk over W+A
    EKW = [wsm.tile([128, WN], BF16, tag='ekwd', name=f'ekw{i}', bufs=16)
           for i in range(NT)]
    EKA = [F(1, i, BS, f'eka{i}') for i in range(NT)]

    def mk_cb_act(tiles, func, bias):
        def cb(jt, ps):
            nc.scalar.activation(tiles[jt][:], ps, func, bias=cv[bias][jt])
        return cb
    gemm('wk', [([t[:] for t in inkW], WN, mk_cb_act(EKW, AF.Exp, 'bk')),
                ([t[:] for t in inkA], BS, mk_cb_act(EKA, AF.Exp, 'bk'))])

    # mixes v, r for A (DVE, during wk)
    invA = [F(0, i, BS, f'iva{i}') for i in range(NT)]
    inrA = [F(4, i, BS, f'ira{i}') for i in range(NT)]
    build_mix(UA, 'mixv', invA, BS)
    build_mix(UA, 'mixr', inrA, BS)

    # wv over W+A
    vvW = [wsm.tile([128, WN], BF16, tag='vvwd', name=f'vvw{i}', bufs=16)
           for i in range(NT)]
    vvA = [F(3, i, BS, f'vva{i}') for i in range(NT)]
    gemm('wv', [([t[:] for t in invW], WN, mk_cb_act(vvW, AF.Identity, 'bv')),
                ([t[:] for t in invA], BS, mk_cb_act(vvA, AF.Identity, 'bv'))])

    # scans W then A (DVE; run under wr)
    wkvW = [Wt(f'wvw{i}') for i in range(NT)]
    wkvA = [F(0, i, BS, f'wva{i}') for i in range(NT)]
    scans(EKW, vvW, wkvW, WN, mask_carry=False)
    scans(EKA, vvA, wkvA, BS, mask_carry=True)  # core0 zeroes carry at W->A

    # wr over W+A; sigmoid fused into wkv*r
    wkvrW = [wsm.tile([128, WN], BF16, tag='wrwd', name=f'wrw{i}', bufs=16)
           for i in range(NT)]
    wkvrA = [F(5, i, BS, f'wra{i}') for i in range(NT)]

    def mk_cb_wr(wkvr, wkv, n, tag):
        def cb(jt, ps):
            rs = scr.tile([128, n], BF16, tag=tag, name=tag, bufs=1,
                          padded_shape=[128, BS])
            nc.scalar.activation(rs[:], ps, AF.Sigmoid, bias=cv['br'][jt])
            nc.vector.tensor_mul(wkvr[jt][:], wkv[jt][:], rs[:])
        return cb
    gemm('wr', [([t[:] for t in inrW], WN, mk_cb_wr(wkvrW, wkvW, WN, 'rsw')),
                ([t[:] for t in inrA], BS, mk_cb_wr(wkvrA, wkvA, BS, 'rsa'))])

    # LN1 + U + mixes for B (DVE, during wr/wk(B))
    sbB, msB = ln_stats([xbB[i][:] for i in range(NT)], BS)
    UB = [upool.tile([128, BS + 1], BF16, tag=f'u{i}', name=f'ub{i}') for i in range(NT)]
    build_U(UB, [xbB[i][:] for i in range(NT)], sbB, msB, Ucol, BS)
    inkB = [F(2, i, BS, f'ikb{i}') for i in range(NT)]
    build_mix(UB, 'mixk', inkB, BS)
    invB = [F(0, i, BS, f'ivb{i}') for i in range(NT)]
    inrB = [F(4, i, BS, f'irb{i}') for i in range(NT)]
    build_mix(UB, 'mixv', invB, BS)
    build_mix(UB, 'mixr', inrB, BS)

    # k/v/r over B
    EKB = [F(1, i, BS, f'ekb{i}') for i in range(NT)]
    gemm('wk', [([t[:] for t in inkB], BS, mk_cb_act(EKB, AF.Exp, 'bk'))])
    vvB = [F(3, i, BS, f'vvb{i}') for i in range(NT)]
    gemm('wv', [([t[:] for t in invB], BS, mk_cb_act(vvB, AF.Identity, 'bv'))])
    wkvB = [F(0, i, BS, f'wvb{i}') for i in range(NT)]
    scans(EKB, vvB, wkvB, BS, mask_carry=False)
    wkvrB = [F(3, i, BS, f'wrb{i}') for i in range(NT)]
    gemm('wr', [([t[:] for t in inrB], BS, mk_cb_wr(wkvrB, wkvB, BS, 'rsa'))])

    # atto over W+A; rz = psum + bo + x; main-block rz spills to DRAM
    rzW = [wsm.tile([128, WN], BF16, tag='rzwd', name=f'rzw{i}', bufs=16)
           for i in range(NT)]
    rzA = [None] * NT
    rzB = [None] * NT

    class Roller:
        """Rolling-prefetch DRAM->SBUF loader: tile jt+DEPTH is allocated
        and its DMA issued at consume(jt), keeping FIFO slot reuse legal
        at small bufs while hiding the DMA under the GEMM panels."""
        DEPTH = 3

        def __init__(self, src_t0):
            self.src_t0 = src_t0
            self.tiles = {}
            for jt in range(self.DEPTH):
                self._fetch(jt)

        def _fetch(self, jt):
            if jt < NT:
                tl = scr.tile([128, BS], BF16, tag='roll', name='roll',
                              bufs=self.DEPTH + 2)
                nc.gpsimd.dma_start(tl[:], self.src(jt))
                self.tiles[jt] = tl

        def src(self, jt):
            return xbTt[jt, :, self.src_t0:self.src_t0 + BS]

        def consume(self, jt):
            self._fetch(jt + self.DEPTH)
            return self.tiles.pop(jt)

    class RzRoller(Roller):
        def __init__(self, rzoff):
            self.rzoff = rzoff
            super().__init__(0)

        def src(self, jt):
            return rzTt[jt, :, self.rzoff:self.rzoff + BS]

    def mk_cb_rzw():
        def cb(jt, ps):
            nc.vector.scalar_tensor_tensor(rzW[jt][:], ps, cv['bo'][jt],
                                           xbW[jt][:], OP.add, OP.add)
        return cb

    def mk_cb_rz(rzlist, x2roll, rzoff):
        def cb(jt, ps):
            rz = rzp.tile([128, BS], BF16, tag='rz', name='rz')
            rzlist[jt] = rz
            x2 = x2roll.consume(jt)
            nc.vector.scalar_tensor_tensor(rz[:], ps, cv['bo'][jt],
                                           x2[:], OP.add, OP.add)
            eng = nc.sync if jt % 2 == 0 else nc.scalar
            eng.dma_start(rzTt[jt, :, rzoff:rzoff + BS], rz[:])
        return cb

    x2A = Roller(A0)
    gemm('wo', [([t[:] for t in wkvrW], WN, mk_cb_rzw()),
                ([t[:] for t in wkvrA], BS, mk_cb_rz(rzA, x2A, 0))])

    # LN2 + U2 for W and A
    sb2W, ms2W = ln_stats([rzW[i][:] for i in range(NT)], WN)
    U2W = [upool.tile([128, WN + 1], BF16, tag=f'u{i}', name=f'u2w{i}',
                      padded_shape=[128, BS + 1]) for i in range(NT)]
    build_U(U2W, [rzW[i][:] for i in range(NT)], sb2W, ms2W, U2col, WN)
    sb2A, ms2A = ln_stats([rzA[i][:] for i in range(NT)], BS)
    U2A = [upool.tile([128, BS + 1], BF16, tag=f'u{i}', name=f'u2a{i}') for i in range(NT)]
    build_U(U2A, [rzA[i][:] for i in range(NT)], sb2A, ms2A, U2col, BS)

    # atto over B
    x2B = Roller(B0)
    gemm('wo', [([t[:] for t in wkvrB], BS, mk_cb_rz(rzB, x2B, BS))])

    # FFN mixes A (before U2B alloc: U2A readers must be emitted first)
    fmkA = [F(0, i, BS, f'fka{i}') for i in range(NT)]
    fmrA = [F(3, i, BS, f'fra{i}') for i in range(NT)]
    build_mix(U2A, 'fmixk', fmkA, BS)
    build_mix(U2A, 'fmixr', fmrA, BS)

    # LN2 + U2 for B
    sb2B, ms2B = ln_stats([rzB[i][:] for i in range(NT)], BS)
    U2B = [upool.tile([128, BS + 1], BF16, tag=f'u{i}', name=f'u2b{i}') for i in range(NT)]
    build_U(U2B, [rzB[i][:] for i in range(NT)], sb2B, ms2B, U2col, BS)

    # FFN block A
    kf2A = [F(1, i, BS, f'k2a{i}') for i in range(NT)]

    def mk_cb_kf(kf2):
        def cb(jt, ps):
            kf = scr.tile([128, BS], BF16, tag='kf', name='kf', bufs=1)
            nc.scalar.activation(kf[:], ps, AF.Identity, bias=cv['bfk'][jt])
            nc.vector.scalar_tensor_tensor(kf2[jt][:], kf[:], 0.0, kf[:],
                                           OP.max, OP.mult)
        return cb
    gemm('wfk', [([t[:] for t in fmkA], BS, mk_cb_kf(kf2A))])

    rfA = [F(5, i, BS, f'rfa{i}') for i in range(NT)]
    gemm('wfr', [([t[:] for t in fmrA], BS, mk_cb_act(rfA, AF.Sigmoid, 'bfr'))])

    # FFN mixes B (during wfk/wfr A)
    fmkB = [F(2, i, BS, f'fkb{i}') for i in range(NT)]
    fmrB = [F(4, i, BS, f'frb{i}') for i in range(NT)]
    build_mix(U2B, 'fmixk', fmkB, BS)
    build_mix(U2B, 'fmixr', fmrB, BS)

    def mk_cb_out(rzroll, rflist, t0):
        def cb(jt, ps):
            t3 = scr.tile([128, BS], BF16, tag='t3o', name='t3o', bufs=1)
            nc.vector.scalar_tensor_tensor(t3[:], ps, cv['bfv'][jt],
                                           rflist[jt][:], OP.add, OP.mult)
            rzl = rzroll.consume(jt)
            ot = scr.tile([128, BS], BF16, tag='ot', name='ot', bufs=2)
            nc.vector.tensor_add(ot[:], t3[:], rzl[:])
            eng = nc.sync if jt % 2 == 0 else nc.scalar
            eng.dma_start(outTt[jt, :, t0:t0 + BS], ot[:])
        return cb
    rzlA = RzRoller(0)
    gemm('wfv', [([t[:] for t in kf2A], BS, mk_cb_out(rzlA, rfA, 0))])

    # FFN block B
    kf2B = [F(0, i, BS, f'k2b{i}') for i in range(NT)]
    gemm('wfk', [([t[:] for t in fmkB], BS, mk_cb_kf(kf2B))])
    rfB = [F(1, i, BS, f'rfb{i}') for i in range(NT)]
    gemm('wfr', [([t[:] for t in fmrB], BS, mk_cb_act(rfB, AF.Sigmoid, 'bfr'))])
    rzlB = RzRoller(BS)
    gemm('wfv', [([t[:] for t in kf2B], BS, mk_cb_out(rzlB, rfB, BS))])


def prep_inputs(inputs):
    f32 = np.float32
    bf16 = ml_dtypes.bfloat16
    x = np.asarray(inputs['x'], f32)
    g1, b1 = np.asarray(inputs['ln1_g'], f32), np.asarray(inputs['ln1_b'], f32)
    g2, b2 = np.asarray(inputs['ln2_g'], f32), np.asarray(inputs['ln2_b'], f32)
    W, Bv = {}, {}
    for key, nm, g, b in [('wk', 'attk', g1, b1), ('wv', 'attv', g1, b1),
                          ('wr', 'attr', g1, b1), ('wfk', 'ffnk', g2, b2),
                          ('wfr', 'ffnr', g2, b2)]:
        w = np.asarray(inputs[nm + '_w'], f32)
        W[key] = (w * g[None, :]).T
        Bv[key] = (np.asarray(inputs[nm + '_b'], f32) + w @ b).astype(f32)
    for key, nm in [('wo', 'atto'), ('wfv', 'ffnv')]:
        w = np.asarray(inputs[nm + '_w'], f32)
        W[key] = w.T
        Bv[key] = np.asarray(inputs[nm + '_b'], f32)
    Wp = {}
    for key, w in W.items():
        wp = w.reshape(NT, 128, NPAN, PW).transpose(0, 2, 1, 3)
        Wp[key] = np.ascontiguousarray(wp.astype(bf16))
    bmap = dict(zip(BNAMES, ['wk', 'wv', 'wr', 'wo', 'wfk', 'wfv', 'wfr']))
    col = lambda a: np.ascontiguousarray(np.asarray(a, f32).reshape(D, 1))
    mixes = {'mixk': inputs['attmixk'], 'mixv': inputs['attmixv'],
             'mixr': inputs['attmixr'], 'fmixk': inputs['ffnmixk'],
             'fmixr': inputs['ffnmixr']}
    ew = np.exp(-np.exp(np.asarray(inputs['time_decay'], f32))).astype(f32)
    eu = np.exp(np.asarray(inputs['time_first'], f32)).astype(f32)
    xt = np.ascontiguousarray(x.T)

    in_maps = []
    for c in range(NCORES):
        s = c * TLOC
        idx = (np.arange(s - H, s + TLOC)) % T
        m = {'xbT': np.ascontiguousarray(xt[:, idx].astype(bf16))}
        for k in WNAMES:
            m[k] = Wp[k]
        for k in BNAMES:
            m[k] = col(Bv[bmap[k]])
        for k, v in mixes.items():
            m[k] = col(v)
        m['onescol'] = np.ones((128, 1), bf16)
        m['onesrow'] = np.ones((1, 128), bf16)
        m['ew'] = col(ew)
        m['eu'] = col(eu)
        m['cmask'] = np.full((D, 1), 0.0 if c == 0 else 1.0, f32)
        in_maps.append(m)
    return in_maps


_CACHED = {}
TRACE = False
LAST = {}


def kernel(**inputs):
    if 'nc' not in _CACHED:
        _CACHED['nc'] = build_kernel()
    nc = _CACHED['nc']
    in_maps = prep_inputs(inputs)
    kw = {}
    if TRACE:
        kw = dict(trace=True, trace_cores=list(range(NCORES)))
    res = run_bass_kernel_spmd(nc, in_maps, list(range(NCORES)), **kw)
    LAST['res'] = res
    parts = [np.asarray(res.results[c]['outT']) for c in range(NCORES)]
    out = np.concatenate(parts, axis=1).T
    return np.ascontiguousarray(out.astype(np.float32))


if __name__ == '__main__':
    import reference
    inputs = {k: np.asarray(v) for k, v in reference.setup_inputs().items()}
    out = kernel(**inputs)
    print('out', out.shape, out.dtype)
